# revision 11
# baseline (speedup 1.0000x reference)
"""Trainium2 Bass kernel for multi-head attention with adaptive span masking.

Computation (per the nn.Module):
    q = (query @ Wq.T) split into B*H rows of size d=64
    attn = softmax((key . q + q @ key_pe) / sqrt(d))
    attn = renormalize(attn * adaptive_span_mask)
    out = (attn . value) merged heads @ Wo.T

Sparsity: the adaptive-span mask is exactly zero for positions
m <= 8159 - span[h]*8192, so those key/value rows contribute nothing
(the only coupling is the 1e-8*sum(softmax) term in the renormalizer,
which is ~1e-7 relative -- far below tolerance). The host inspects the
span input, derives a per-head start offset S_h (aligned down to 128),
and the device kernel only loads/computes the [S_h, 8192) tail.

Sharding: batch-parallel across 8 cores. Core c gets batches [4c, 4c+4)
(all 8 heads) = rows [32c, 32c+32) of key/value; Wq/Wo/key_pe/span are
replicated. Each core produces its own [4, 512] output block; the host
concatenates. No collectives needed. Per-core DMA bytes are identical
across cores (each core has all 8 heads), so the sharding stays
balanced under the per-head sparsity.

Schedule: rows are processed head-major (heads ordered big->small so
the pipeline fills early and drains on the tiny head), with each head's
positional scores + mask computed inline right before its 4 rows --
the kv DMA stream stays busy from t=0 instead of stalling behind a
monolithic setup phase. PV matmuls are quad-batched (4 m-blocks per
instruction, 256 streamed columns) with a diagonal fix-up.
"""

import math
import os
import sys

import numpy as np

for _p in ("/opt/trn_rl_repo", "/root/.axon_site/_ro/trn_rl_repo"):
    if os.path.isdir(_p) and _p not in sys.path:
        sys.path.insert(0, _p)

import concourse.bass as bass
import concourse.bacc as bacc
import concourse.mybir as mybir
from concourse.bass import ts
from concourse.masks import make_identity
from concourse.tile import TileContext

F32 = mybir.dt.float32

# Problem constants (hardcoded per contest contract)
NHEADS = 8
HEAD_DIM = 64
HID = NHEADS * HEAD_DIM  # 512
B = 32
M = 8192
RAMP = 32.0

N_CORES = 8
BPC = B // N_CORES        # 4 batches per core
NPC = BPC * NHEADS        # 32 (b,h) rows per core

_CACHE = {}


def _head_starts(span):
    """Per-head first position with a (possibly) nonzero mask, aligned
    down to a multiple of 128. mask[m] = clip((m-8191+span*8192)/32+1,0,1)
    is zero for m <= 8159 - span*8192."""
    span = np.asarray(span, np.float64).ravel()
    starts = []
    for h in range(NHEADS):
        m_min = int(np.floor(8159.0 - span[h] * M))
        m_min -= 2  # safety vs f32 rounding on device
        s = max(0, (m_min // 128) * 128)
        starts.append(s)
    return tuple(starts)


def build_nc(starts):
    nc = bacc.Bacc(None, target_bir_lowering=False)
    AF = mybir.ActivationFunctionType
    ALU = mybir.AluOpType
    BF16 = mybir.dt.bfloat16

    MOH = [(M - s) // 128 for s in starts]      # per-head m-blocks
    OFF = [0]
    for h in range(NHEADS):
        OFF.append(OFF[h] + MOH[h])
    TOT = OFF[NHEADS]                            # total m-blocks, all heads
    MOMAX = max(MOH)
    SMIN = min(starts)
    # head order: big first (pipeline fill), smallest last (short drain)
    HORD = sorted(range(NHEADS), key=lambda h: -MOH[h])

    q_d = nc.dram_tensor("query", [BPC, HID], F32, kind="ExternalInput")
    k_d = nc.dram_tensor("key", [NPC, M, HEAD_DIM], F32, kind="ExternalInput")
    v_d = nc.dram_tensor("value", [NPC, M, HEAD_DIM], F32, kind="ExternalInput")
    wq_d = nc.dram_tensor("Wq", [HID, HID], F32, kind="ExternalInput")
    wo_d = nc.dram_tensor("Wo", [HID, HID], F32, kind="ExternalInput")
    kpe_d = nc.dram_tensor("key_pe", [HEAD_DIM, M], F32, kind="ExternalInput")
    span_d = nc.dram_tensor("span", [NHEADS, 1], F32, kind="ExternalInput")
    out_d = nc.dram_tensor("out", [BPC, HID], F32, kind="ExternalOutput")

    with TileContext(nc) as tc:
        with (
            tc.tile_pool(name="persist", bufs=1) as persist,
            # main-loop pools created BEFORE setup pools so the kv DMAs get
            # SBUF ranges disjoint from setup tiles (no WAR dep -> kv loads
            # start at t=0, overlapping the whole setup phase)
            tc.tile_pool(name="kv", bufs=3) as kv_pool,
            tc.tile_pool(name="sc", bufs=3) as sc_pool,
            tc.tile_pool(name="fin", bufs=1) as fin_pool,
        ):
            identity = persist.tile([128, 128], F32, tag="identity")
            make_identity(nc, identity[:])
            ones_row = persist.tile([1, 128], F32, tag="ones_row")
            nc.vector.memset(ones_row[:], 1.0)
            ones_col = persist.tile([128, 1], F32, tag="ones_col")
            nc.vector.memset(ones_col[:], 1.0)

            NQMAX = (MOMAX + 3) // 4
            # w_sc: per-row PV weights scattered so quad q's 4 vectors sit
            # at columns {0,32,64,96} of block q -> matmul output partitions
            # {0,32,64,96} (legal engine partition bases). Zero elsewhere;
            # only the scattered columns are ever (re)written, so one
            # upfront memset per rotating buffer suffices.
            WSC_BUFS = 3
            w_sc = [
                persist.tile([128, NQMAX, 128], mybir.dt.bfloat16,
                             name=f"w_sc{j}", tag=f"w_sc{j}")
                for j in range(WSC_BUFS)
            ]
            for j in range(WSC_BUFS):
                nc.vector.memset(w_sc[j][:], 0.0)

            woT = [persist.tile([128, HID], F32, name=f"woT{j}", tag=f"woT{j}") for j in range(4)]
            q_sb = persist.tile([BPC, HID], F32, tag="q_sb")
            qts = persist.tile([HEAD_DIM, BPC, NHEADS], F32, tag="qts")
            qrep = persist.tile([128, BPC, HID], F32, tag="qrep")
            pos_all = persist.tile([128, TOT, BPC], F32, tag="pos_all")
            masks = persist.tile([128, TOT], F32, tag="masks")
            ao_sb = persist.tile([1, BPC, HID], F32, tag="ao_sb")
            span_b = persist.tile([128, NHEADS], F32, tag="span_b")
            kpe_sb = persist.tile([HEAD_DIM, M - SMIN], F32, tag="kpe_sb")

            # ---------------- setup: span bias, q, qts, qrep ----------------
            with (
                tc.tile_pool(name="setupA", bufs=1) as sa,
                tc.tile_pool(name="psA", bufs=2, space="PSUM") as psA,
            ):
                # span broadcast + affine -> span_b[p, h] (mask bias per head)
                span_row = sa.tile([1, NHEADS], F32, tag="span_row")
                nc.sync.dma_start(out=span_row[:], in_=span_d[:].rearrange("h o -> o h"))
                ps_sp = psA.tile([128, NHEADS], F32, tag="ps_sp", bufs=1)
                nc.tensor.matmul(
                    ps_sp[:], ones_row[:], span_row[:], start=True, stop=True
                )
                bias_const = float(-(M - 1) / RAMP + 1.0)  # -254.96875
                nc.scalar.activation(
                    out=span_b[:], in_=ps_sp[:], func=AF.Copy,
                    scale=float(M / RAMP), bias=bias_const,
                )

                wqT = [sa.tile([128, HID], F32, name=f"wqT{j}", tag=f"wqT{j}") for j in range(4)]
                wq_sb = [sa.tile([128, HID], F32, name=f"wq_sb{i}", tag="wq_sb", bufs=2) for i in range(4)]
                for i in range(4):
                    nc.sync.dma_start(out=wq_sb[i][:], in_=wq_d[ts(i, 128), :])
                query_sb = sa.tile([BPC, HID], F32, tag="query_sb")
                nc.sync.dma_start(out=query_sb[:], in_=q_d[:])
                # key_pe: only the union-sparse tail is ever used
                nc.sync.dma_start(out=kpe_sb[:], in_=kpe_d[:, SMIN:])
                for io in range(4):
                    for jo in range(4):
                        pwt = psA.tile([128, 128], F32, tag="pwt")
                        nc.tensor.matmul(
                            pwt[:], wq_sb[io][:, ts(jo, 128)], identity[:],
                            start=True, stop=True,
                        )
                        nc.scalar.copy(wqT[jo][:, ts(io, 128)], pwt[:])

                qTq = [sa.tile([128, BPC], F32, name=f"qTq{j}", tag=f"qTq{j}") for j in range(4)]
                for jo in range(4):
                    pqt = psA.tile([128, BPC], F32, tag="pwt")
                    nc.tensor.matmul(
                        pqt[:], query_sb[:, ts(jo, 128)], identity[0:BPC, 0:BPC],
                        start=True, stop=True,
                    )
                    nc.scalar.copy(qTq[jo][:], pqt[:])
                # q = query @ Wq.T  ->  [4, 512]
                ps_q = psA.tile([BPC, HID], F32, tag="ps_q", bufs=1)
                for jo in range(4):
                    nc.tensor.matmul(
                        ps_q[:], qTq[jo][:], wqT[jo][:],
                        start=(jo == 0), stop=(jo == 3),
                    )
                nc.scalar.copy(q_sb[:], ps_q[:])
                # qts[d, b, h] = q[b, h*64+d]   (64 partitions)
                for h in range(NHEADS):
                    pqh = psA.tile([HEAD_DIM, BPC], F32, tag="pwt")
                    nc.tensor.matmul(
                        pqh[:], q_sb[:, ts(h, HEAD_DIM)], identity[0:BPC, 0:BPC],
                        start=True, stop=True,
                    )
                    nc.scalar.copy(qts[:, :, h], pqh[:])
                # qrep[p, b, :] = q[b, :] for all 128 p
                # (bounce via DRAM -- DMA partition-broadcast needs a DRAM src)
                with tc.tile_pool(name="dramq", bufs=1, space="DRAM") as dq:
                    q_dram = dq.tile([BPC, HID], F32, tag="q_dram")
                    nc.sync.dma_start(out=q_dram[:], in_=q_sb[:])
                    for b in range(BPC):
                        nc.gpsimd.dma_start(
                            out=qrep[:, b, :],
                            in_=q_dram[b : b + 1, :].partition_broadcast(128),
                        )

            # Wo load + transpose are only needed by the final projection;
            # issued late (program order) so they don't delay kv DMAs.
            wo_sb = [fin_pool.tile([128, HID], F32, name=f"wo_sb{i}", tag=f"wo_sb{i}") for i in range(4)]

            with (
                tc.tile_pool(name="ps_s", bufs=1, space="PSUM") as ps_s_pool,
                tc.tile_pool(name="ps_o", bufs=2, space="PSUM") as ps_o_pool,
                tc.tile_pool(name="ps_pos", bufs=2, space="PSUM") as ps_pos_pool,
            ):
                # ------------- main loop: head-major, setup inlined ---------
                for hi, h in enumerate(HORD):
                    moh = MOH[h]
                    off = OFF[h]
                    s_h = starts[h]
                    # --- per-head positional scores (PE) ---
                    # pos[p, off+mo, b] =
                    #     sum_d key_pe[d, S_h + p*moh + mo] * q[(b,h), d]
                    kpe_r = kpe_sb[:, s_h - SMIN :].rearrange(
                        "d (p mo) -> d mo p", mo=moh
                    )
                    mo = 0
                    while mo < moh:
                        g = min(16, moh - mo)
                        ps_p = ps_pos_pool.tile([128, 16, BPC], F32, tag="ps_p")
                        for k in range(g):
                            nc.tensor.matmul(
                                ps_p[:, k, :], kpe_r[:, mo + k, :],
                                qts[:, :, h],
                                start=True, stop=True,
                            )
                        nc.scalar.copy(
                            pos_all[:, off + mo : off + mo + g, :],
                            ps_p[:, 0:g, :],
                        )
                        mo += g
                    # --- per-head mask ---
                    # masks[p, off+mo] = clip((S_h + p*moh + mo)/32
                    #                         + span[h]*256 - 254.96875, 0, 1)
                    m_f = sc_pool.tile([128, MOMAX], F32, tag="m_f", bufs=1)
                    nc.gpsimd.iota(
                        out=m_f[:, 0:moh],
                        pattern=[[1, moh]], base=s_h,
                        channel_multiplier=moh,
                        allow_small_or_imprecise_dtypes=True,
                    )
                    nc.scalar.activation(
                        out=masks[:, off : off + moh],
                        in_=m_f[:, 0:moh],
                        func=AF.Identity,
                        scale=float(1.0 / RAMP), bias=span_b[:, h : h + 1],
                    )
                    nc.vector.tensor_scalar(
                        out=masks[:, off : off + moh],
                        in0=masks[:, off : off + moh],
                        scalar1=0.0, scalar2=1.0,
                        op0=ALU.max, op1=ALU.min,
                    )

                    # --- the head's 4 batch rows ---
                    for b in range(BPC):
                        i = b * NHEADS + h
                        kt = kv_pool.tile([128, MOMAX, HEAD_DIM], F32, tag="kt")
                        vt = kv_pool.tile([128, MOMAX, HEAD_DIM], F32, tag="vt")
                        nc.sync.dma_start(
                            out=kt[:, 0:moh, :],
                            in_=k_d[i, s_h:, :].rearrange(
                                "(p mo) d -> p mo d", p=128
                            ),
                        )
                        nc.scalar.dma_start(
                            out=vt[:, 0:moh, :],
                            in_=v_d[i, s_h:, :].rearrange(
                                "(p mo) d -> p mo d", p=128
                            ),
                        )
                        # f32 -> bf16 cast on the scalar engine; bf16 halves
                        # PE work in the PV matmuls
                        vtb = kv_pool.tile([128, MOMAX, HEAD_DIM], BF16, tag="vtb")
                        nc.scalar.copy(vtb[:, 0:moh, :], vt[:, 0:moh, :])
                        # content + positional scores
                        prod = sc_pool.tile([128, MOMAX, HEAD_DIM], F32, tag="prod", bufs=1)
                        q_b = (
                            qrep[:, b, ts(h, HEAD_DIM)]
                            .rearrange("p (x d) -> p x d", x=1)
                            .broadcast_to((128, moh, HEAD_DIM))
                        )
                        nc.vector.tensor_mul(prod[:, 0:moh, :], kt[:, 0:moh, :], q_b)
                        scores = sc_pool.tile([128, MOMAX], F32, tag="scores")
                        nc.vector.reduce_sum(
                            scores[:, 0:moh], prod[:, 0:moh, :],
                            axis=mybir.AxisListType.X,
                        )
                        nc.vector.tensor_add(
                            scores[:, 0:moh], scores[:, 0:moh],
                            pos_all[:, off : off + moh, b],
                        )
                        # e = exp(scores / sqrt(d)), Sigma_e fused
                        e_t = sc_pool.tile([128, MOMAX], F32, tag="e_t")
                        sums = sc_pool.tile([128, 2], F32, tag="sums")
                        nc.scalar.activation(
                            out=e_t[:, 0:moh], in_=scores[:, 0:moh], func=AF.Exp,
                            scale=float(1.0 / math.sqrt(HEAD_DIM)),
                            accum_out=sums[:, 0:1],
                        )
                        # w = e * mask[h] (bf16 for the PE), then Sigma_w
                        w_t = sc_pool.tile([128, MOMAX], BF16, tag="w_t")
                        nc.vector.tensor_mul(
                            w_t[:, 0:moh], e_t[:, 0:moh],
                            masks[:, off : off + moh],
                        )
                        nc.vector.reduce_sum(
                            sums[:, 1:2], w_t[:, 0:moh],
                            axis=mybir.AxisListType.X,
                        )
                        # partition-reduce both sums: [1, 2] = ones.T @ sums
                        ps_s = ps_s_pool.tile([1, 2], F32, tag="ps_s")
                        nc.tensor.matmul(
                            ps_s[:], ones_col[:], sums[:], start=True, stop=True
                        )
                        sums_sb = sc_pool.tile([1, 2], F32, tag="sums_sb")
                        nc.scalar.copy(sums_sb[:], ps_s[:])
                        # u = Sigma_w + 1e-8 * Sigma_e ; scal = 1/u
                        u_t = sc_pool.tile([1, 1], F32, tag="u_t")
                        nc.scalar.activation(
                            out=u_t[:], in_=sums_sb[:, 0:1], func=AF.Identity,
                            scale=1e-8, bias=sums_sb[:, 1:2],
                        )
                        scal = sc_pool.tile([1, 1], F32, tag="scal")
                        nc.vector.reciprocal(scal[:], u_t[:])
                        # out_row = sum_m w[m] * value[m, :]
                        # quad-batched PV: scatter quad q's 4 w vectors to
                        # columns {0,32,64,96} of w_sc block q; one matmul
                        # per quad streams v for the 4 blocks (256 cols)
                        # into psum [128, 256]. The valid sums land on the
                        # "diagonal" cells (partition 32j, cols 64j:64j+64)
                        # -- legal engine partition bases. Other cells are
                        # never read (incl. stale-weight lanes of partial
                        # quads: partition/32 == col/64 only on the diag).
                        wsc = w_sc[(hi * BPC + b) % WSC_BUFS]
                        wv4 = wsc[:].rearrange(
                            "p q (j x) -> p q j x", j=4, x=32
                        )[:, :, :, 0]
                        nq_full = moh // 4
                        rem = moh - 4 * nq_full
                        if nq_full > 0:
                            nc.scalar.copy(
                                wv4[:, 0:nq_full, :],
                                w_t[:, 0 : 4 * nq_full].rearrange(
                                    "p (q j) -> p q j", j=4
                                ),
                            )
                        if rem > 0:
                            nc.scalar.copy(
                                wv4[:, nq_full, 0:rem],
                                w_t[:, 4 * nq_full : moh],
                            )
                        qn = min(4, moh)
                        ps_o = ps_o_pool.tile([128, 256], F32, tag="ps_o")
                        mo = 0
                        first = True
                        while mo < moh:
                            g = min(4, moh - mo)
                            nc.tensor.matmul(
                                ps_o[:, 0 : g * 64],
                                wsc[:, mo // 4, :],
                                vtb[:, mo : mo + g, :],
                                start=first,
                                stop=(mo + g >= moh),
                            )
                            first = False
                            mo += g
                        # diagonal fix-up: pull the qn valid [1, 64] sums
                        # (partition bases 0/32/64/96), then tree-add
                        accs = []
                        for j in range(qn):
                            acc_j = sc_pool.tile([1, HEAD_DIM], F32, tag=f"acc{j}")
                            nc.scalar.copy(
                                acc_j[:],
                                ps_o[32 * j : 32 * j + 1, ts(j, HEAD_DIM)],
                            )
                            accs.append(acc_j)
                        for j in range(1, qn):
                            nc.vector.tensor_add(
                                accs[0][:], accs[0][:], accs[j][:]
                            )
                        # ao[0, b, h*64:(h+1)*64] = acc * scal
                        nc.scalar.activation(
                            out=ao_sb[0:1, b, ts(h, HEAD_DIM)], in_=accs[0][:],
                            func=AF.Copy, scale=scal[:, 0:1],
                        )

                    # late Wo load: after the first head group, overlapped
                    # with the remaining rows, before the final projection
                    if hi == 0:
                        for wi in range(4):
                            nc.scalar.dma_start(out=wo_sb[wi][:], in_=wo_d[ts(wi, 128), :])
                    if hi == 1:
                        for io in range(4):
                            for jo in range(4):
                                pwt2 = ps_pos_pool.tile([128, 128], F32, tag="pwt2", bufs=1)
                                nc.tensor.matmul(
                                    pwt2[:], wo_sb[io][:, ts(jo, 128)], identity[:],
                                    start=True, stop=True,
                                )
                                nc.scalar.copy(woT[jo][:, ts(io, 128)], pwt2[:])

                # ---------------- output projection -------------------------
                aoT = []
                for co in range(4):
                    ps_t2 = ps_pos_pool.tile([128, BPC], F32, name="ps_t2", tag="ps_fin", bufs=1)
                    for b in range(BPC):
                        nc.tensor.matmul(
                            ps_t2[:, b : b + 1],
                            ao_sb[0:1, b, ts(co, 128)],
                            identity[0:1, 0:1],
                            start=True, stop=True,
                        )
                    t_sb = fin_pool.tile([128, BPC], F32, name=f"t_sb{co}", tag=f"t_sb{co}")
                    nc.scalar.copy(t_sb[:], ps_t2[:])
                    aoT.append(t_sb)
                ps_f = ps_pos_pool.tile([BPC, HID], F32, name="ps_f", tag="ps_fin", bufs=1)
                for co in range(4):
                    nc.tensor.matmul(
                        ps_f[:], aoT[co][:], woT[co][:],
                        start=(co == 0), stop=(co == 3),
                    )
                out_sb = fin_pool.tile([BPC, HID], F32, tag="out_sb")
                nc.scalar.copy(out_sb[:], ps_f[:])
                nc.sync.dma_start(out=out_d[:], in_=out_sb[:])

    nc.compile()
    return nc


def _get_nc(starts):
    if starts not in _CACHE:
        _CACHE[starts] = build_nc(starts)
    return _CACHE[starts]


def _make_in_maps(query, key, value, Wq, Wo, key_pe, span):
    q2 = np.ascontiguousarray(np.asarray(query, np.float32).reshape(B, HID))
    key = np.asarray(key, np.float32)
    value = np.asarray(value, np.float32)
    Wq = np.ascontiguousarray(np.asarray(Wq, np.float32))
    Wo = np.ascontiguousarray(np.asarray(Wo, np.float32))
    key_pe = np.ascontiguousarray(np.asarray(key_pe, np.float32))
    span = np.ascontiguousarray(np.asarray(span, np.float32))
    in_maps = []
    for c in range(N_CORES):
        in_maps.append(
            {
                "query": np.ascontiguousarray(q2[c * BPC : (c + 1) * BPC]),
                "key": np.ascontiguousarray(key[c * NPC : (c + 1) * NPC]),
                "value": np.ascontiguousarray(value[c * NPC : (c + 1) * NPC]),
                "Wq": Wq,
                "Wo": Wo,
                "key_pe": key_pe,
                "span": span,
            }
        )
    return in_maps


def _install_ntff_hook():
    """Shim antenv.axon_hooks with a ctypes NTFF profile hook so
    run_bass_kernel_spmd(trace=True) works in this container."""
    import contextlib
    import ctypes
    import types

    try:
        import antenv.axon_hooks  # noqa: F401

        return
    except ImportError:
        pass
    so_path = "/opt/axon/libaxon_pjrt.so"
    import antenv

    mod = types.ModuleType("antenv.axon_hooks")
    holder = {"hook": None}

    if os.path.exists(so_path):
        lib = ctypes.CDLL(so_path)
        if hasattr(lib, "axon_start_nrt_profile"):
            lib.axon_start_nrt_profile.argtypes = [
                ctypes.POINTER(ctypes.c_int64),
                ctypes.c_size_t,
            ]
            lib.axon_start_nrt_profile.restype = ctypes.c_int64
            lib.axon_stop_nrt_profile.argtypes = [ctypes.c_char_p]
            lib.axon_stop_nrt_profile.restype = ctypes.c_int64

            @contextlib.contextmanager
            def _hook(output_dir, device_ids):
                import jax

                jax.devices()
                if device_ids:
                    ids = (ctypes.c_int64 * len(device_ids))(*device_ids)
                    rc = lib.axon_start_nrt_profile(ids, len(device_ids))
                else:
                    rc = lib.axon_start_nrt_profile(None, 0)
                if rc != 0:
                    raise RuntimeError(f"axon_start_nrt_profile rc={rc}")
                try:
                    yield
                finally:
                    n = lib.axon_stop_nrt_profile(str(output_dir).encode())
                    print(f"profile: {n} file(s) written to {output_dir}")

            holder["hook"] = _hook

    mod.get_axon_ntff_profile_hook = lambda: holder["hook"]
    mod.set_axon_ntff_profile_hook = lambda h: holder.__setitem__("hook", h)
    sys.modules["antenv.axon_hooks"] = mod
    antenv.axon_hooks = mod


def run(query, key, value, Wq, Wo, key_pe, span, trace=False):
    """Run on hardware; returns (output [B,1,HID], BassKernelResults)."""
    from concourse import bass_utils
    from concourse.bass_utils import run_bass_kernel_spmd

    if trace:
        _install_ntff_hook()
        bass_utils.upload_artifacts = lambda tmpdir: f"local:{tmpdir}"
    nc = _get_nc(_head_starts(span))
    in_maps = _make_in_maps(query, key, value, Wq, Wo, key_pe, span)
    res = run_bass_kernel_spmd(nc, in_maps, list(range(N_CORES)), trace=trace)
    out = np.concatenate(
        [np.asarray(res.results[c]["out"]) for c in range(N_CORES)], axis=0
    )
    return out.reshape(B, 1, HID).astype(np.float32), res


def kernel(query, key, value, Wq, Wo, key_pe, span):
    out, _ = run(query, key, value, Wq, Wo, key_pe, span, trace=False)
    return out
